# revision 3
# baseline (speedup 1.0000x reference)
"""Trainium2 Bass kernel for nn_BitwiseMLP: 3x (Linear + training-mode BatchNorm).

Math: reference computes, per layer,  h = gamma * (y - mean_B(y)) * rsqrt(var_B(y) + eps) + beta
with y = x @ W.T + b.  BatchNorm is invariant to per-feature constant shifts of y, so
  - every linear bias b_l cancels exactly,
  - the additive part of each BN affine (beta_l - a_l*mean_l) feeds the next linear as a
    per-feature constant -> also cancels under the next BN.
Only the multiplicative scales a_l = gamma_l * rsqrt(var_l + eps) propagate (folded into the
next layer's input activations), plus one final affine a2*u2 + (beta2 - a2*mean2) on the output.

Device layout: everything transposed -> activations are [features, batch_rows] so BN stats are
free-axis reductions and scales are per-partition multiplies. Batch is sharded 8 ways
(2048 rows/core); weights replicated. Matmuls in bf16 (fp32 PSUM accumulate), stats fp32,
cross-core stats via one small AllReduce per layer.
"""

import numpy as np
import ml_dtypes

# ---- problem constants (full size; hardcoded per harness contract) ----
N_CORES = 8
B_FULL = 16384
D_IN = 1024
D_H = 2048
D_OUT = 1024
BN_EPS = 1e-5

_PROG_CACHE = {}
LAST_RESULTS = None  # BassKernelResults of the most recent run (for test harness)


def build_program(R, B_total):
    """Build the per-core Bass program. R = batch rows per core (multiple of 512)."""
    import concourse.bacc as bacc
    import concourse.mybir as mybir
    import concourse.tile as tile

    f32 = mybir.dt.float32
    bf16 = mybir.dt.bfloat16
    Alu = mybir.AluOpType
    Act = mybir.ActivationFunctionType

    NT = R // 512  # n-chunks of 512 rows
    assert R % 512 == 0
    KT = [D_IN // 128, D_H // 128, D_H // 128]  # k-tiles per layer
    MT = [D_H // 128, D_H // 128, D_OUT // 128]  # m-strips per layer
    inv_B = 1.0 / float(B_total)
    GROUP = [list(range(N_CORES))]

    nc = bacc.Bacc(None, num_devices=N_CORES)

    xt_d = nc.dram_tensor("xt", [D_IN, R], bf16, kind="ExternalInput")
    w0_d = nc.dram_tensor("w0t", [D_IN, D_H], bf16, kind="ExternalInput")
    w1_d = nc.dram_tensor("w1t", [D_H, D_H], bf16, kind="ExternalInput")
    w2_d = nc.dram_tensor("w2t", [D_H, D_OUT], bf16, kind="ExternalInput")
    g0_d = nc.dram_tensor("g0", [D_H], f32, kind="ExternalInput")
    g1_d = nc.dram_tensor("g1", [D_H], f32, kind="ExternalInput")
    g2_d = nc.dram_tensor("g2", [D_OUT], f32, kind="ExternalInput")
    b2_d = nc.dram_tensor("beta2", [D_OUT], f32, kind="ExternalInput")
    out_d = nc.dram_tensor("out", [D_OUT, R], f32, kind="ExternalOutput")

    cc_in = [nc.dram_tensor(f"cc_in{l}", [128, 2 * MT[l]], f32) for l in range(3)]
    cc_out = [nc.dram_tensor(f"cc_out{l}", [128, 2 * MT[l]], f32) for l in range(3)]

    with tile.TileContext(nc) as tc:
        import contextlib

        with contextlib.ExitStack() as ctx:
            big = ctx.enter_context(tc.tile_pool(name="big", bufs=4))
            wpool = ctx.enter_context(tc.tile_pool(name="wstrip", bufs=3))
            pspool = ctx.enter_context(tc.tile_pool(name="psum", bufs=8, space="PSUM"))
            small = ctx.enter_context(tc.tile_pool(name="small", bufs=1))
            scratch = ctx.enter_context(tc.tile_pool(name="scratch", bufs=2))

            # ---- constants / per-feature params ----
            eps_t = small.tile([128, 1], f32, tag="eps")
            nc.vector.memset(eps_t, BN_EPS)
            g_t = []
            for l, gd in enumerate((g0_d, g1_d, g2_d)):
                t = small.tile([128, MT[l]], f32, tag=f"g{l}")
                nc.sync.dma_start(out=t, in_=gd[:].rearrange("(m p) -> p m", p=128))
                g_t.append(t)
            b2_t = small.tile([128, MT[2]], f32, tag="b2")
            nc.sync.dma_start(out=b2_t, in_=b2_d[:].rearrange("(m p) -> p m", p=128))

            # ---- resident loads: xT and W0T (per k-tile DMAs) ----
            xt = big.tile([128, KT[0], R], bf16, tag="big")
            w0 = big.tile([128, KT[0], D_H], bf16, tag="big")
            xt_r = xt_d[:].rearrange("(j p) r -> p j r", p=128)
            w0_r = w0_d[:].rearrange("(j p) f -> p j f", p=128)
            for j in range(KT[0]):
                nc.sync.dma_start(out=xt[:, j, :], in_=xt_r[:, j, :])
                nc.sync.dma_start(out=w0[:, j, :], in_=w0_r[:, j, :])

            def u_pair(pool_tag, halves, dtype, strips_per_half):
                return [
                    big.tile(
                        [128, strips_per_half, R],
                        dtype,
                        tag="big",
                        name=f"{pool_tag}_{h}",
                    )
                    for h in range(halves)
                ]

            def u_slice(pair, strips_per_half, j, n=None):
                t = pair[j // strips_per_half]
                jj = j % strips_per_half
                if n is None:
                    return t[:, jj, :]
                return t[:, jj, n * 512 : (n + 1) * 512]

            def stats_block(l, SQ, want_c, beta_t):
                """sum/sumsq partials -> allreduce -> a (= gamma*rsqrt(var+eps)) [, c]."""
                mt = MT[l]
                sf = small.tile([128, 2, mt], f32, tag=f"sf{l}")
                sqv = SQ.rearrange("p (s m n) -> p s m n", s=2, n=NT)
                nc.vector.tensor_reduce(
                    out=sf[:, 0, :], in_=sqv[:, 0], axis=mybir.AxisListType.X, op=Alu.add
                )
                nc.vector.tensor_reduce(
                    out=sf[:, 1, :], in_=sqv[:, 1], axis=mybir.AxisListType.X, op=Alu.add
                )
                nc.sync.dma_start(out=cc_in[l][:], in_=sf)
                nc.gpsimd.collective_compute(
                    "AllReduce",
                    Alu.add,
                    replica_groups=GROUP,
                    ins=[cc_in[l][:]],
                    outs=[cc_out[l][:]],
                )
                sg = small.tile([128, 2, mt], f32, tag=f"sg{l}")
                nc.sync.dma_start(
                    out=sg, in_=cc_out[l][:].rearrange("p (s m) -> p s m", s=2)
                )
                mean = small.tile([128, mt], f32, tag=f"mean{l}")
                var = small.tile([128, mt], f32, tag=f"var{l}")
                tmp = small.tile([128, mt], f32, tag=f"tmp{l}")
                nc.vector.tensor_scalar_mul(mean, sg[:, 0, :], inv_B)
                nc.vector.tensor_scalar_mul(var, sg[:, 1, :], inv_B)
                nc.vector.tensor_mul(tmp, mean, mean)
                nc.vector.tensor_sub(var, var, tmp)
                # var <- sqrt(var + eps), then reciprocal -> rstd
                nc.scalar.activation(out=var, in_=var, func=Act.Sqrt, bias=eps_t[:, 0:1])
                nc.vector.reciprocal(out=var, in_=var)
                a = small.tile([128, mt], f32, tag=f"a{l}")
                nc.vector.tensor_mul(a, var, g_t[l])
                if not want_c:
                    return a, None
                c = small.tile([128, mt], f32, tag=f"c{l}")
                nc.vector.tensor_mul(tmp, a, mean)
                nc.vector.tensor_sub(c, beta_t, tmp)
                return a, c

            def layer(l, lhs_getter, rhs_pair, rhs_sph, dest_pair, dest_sph, dest_dt):
                """One linear layer: dest = rhs^T-layout matmul, plus sum/sumsq stats."""
                SQ = small.tile([128, 2 * MT[l] * NT], f32, tag=f"SQ{l}")
                for m in range(MT[l]):
                    lhs = lhs_getter(m)
                    for n in range(NT):
                        ps = pspool.tile([128, 512], f32, tag="ps")
                        for j in range(KT[l]):
                            nc.tensor.matmul(
                                ps,
                                lhs(j),
                                u_slice(rhs_pair, rhs_sph, j, n),
                                start=(j == 0),
                                stop=(j == KT[l] - 1),
                            )
                        dest = u_slice(dest_pair, dest_sph, m, n)
                        idx = m * NT + n
                        nc.scalar.activation(
                            out=dest,
                            in_=ps,
                            func=Act.Copy,
                            accum_out=SQ[:, idx : idx + 1],
                        )
                        sc = scratch.tile([128, 512], bf16, tag="sqsc")
                        qidx = MT[l] * NT + idx
                        nc.scalar.activation(
                            out=sc,
                            in_=ps,
                            func=Act.Square,
                            accum_out=SQ[:, qidx : qidx + 1],
                        )
                return SQ

            # ================= layer 0 =================
            u0 = u_pair("u0", 2, bf16, MT[0] // 2)

            def lhs0(m):
                return lambda j: w0[:, j, m * 128 : (m + 1) * 128]

            SQ0 = layer(0, lhs0, [xt], KT[0], u0, MT[0] // 2, bf16)
            a0, _ = stats_block(0, SQ0, False, None)
            for j in range(KT[1]):
                s = u_slice(u0, MT[0] // 2, j)
                nc.vector.tensor_scalar_mul(s, s, a0[:, j : j + 1])

            # ================= layer 1 =================
            u1 = u_pair("u1", 2, bf16, MT[1] // 2)

            def lhs_strip(w_dram):
                def getter(m):
                    w = wpool.tile([128, KT[1], 128], bf16, tag="w")
                    nc.sync.dma_start(
                        out=w,
                        in_=w_dram[:][:, m * 128 : (m + 1) * 128].rearrange(
                            "(j p) f -> p j f", p=128
                        ),
                    )
                    return lambda j: w[:, j, :]

                return getter

            SQ1 = layer(1, lhs_strip(w1_d), u0, MT[0] // 2, u1, MT[1] // 2, bf16)
            a1, _ = stats_block(1, SQ1, False, None)
            for j in range(KT[2]):
                s = u_slice(u1, MT[1] // 2, j)
                nc.vector.tensor_scalar_mul(s, s, a1[:, j : j + 1])

            # ================= layer 2 =================
            u2 = u_pair("u2", 2, f32, MT[2] // 2)
            SQ2 = layer(2, lhs_strip(w2_d), u1, MT[1] // 2, u2, MT[2] // 2, f32)
            a2, c2 = stats_block(2, SQ2, True, b2_t)

            # ---- final affine + writeout ----
            for m in range(MT[2]):
                s = u_slice(u2, MT[2] // 2, m)
                nc.vector.tensor_scalar(
                    s, s, a2[:, m : m + 1], c2[:, m : m + 1], Alu.mult, Alu.add
                )
                nc.sync.dma_start(out=out_d[m * 128 : (m + 1) * 128, :], in_=s)

    nc.compile()
    return nc


def _get_program(R, B_total):
    key = (R, B_total)
    if key not in _PROG_CACHE:
        _PROG_CACHE[key] = build_program(R, B_total)
    return _PROG_CACHE[key]


def prep_inputs(x, W0, W1, W2, gamma0, gamma1, gamma2, beta2, n_cores=N_CORES):
    """Host-side: transpose, cast to bf16, shard batch columns."""
    bf = ml_dtypes.bfloat16
    xT = np.ascontiguousarray(x.T)  # [D_IN, B]
    R = x.shape[0] // n_cores
    w0t = np.ascontiguousarray(W0.T).astype(bf)
    w1t = np.ascontiguousarray(W1.T).astype(bf)
    w2t = np.ascontiguousarray(W2.T).astype(bf)
    g0 = np.ascontiguousarray(gamma0, dtype=np.float32)
    g1 = np.ascontiguousarray(gamma1, dtype=np.float32)
    g2 = np.ascontiguousarray(gamma2, dtype=np.float32)
    b2 = np.ascontiguousarray(beta2, dtype=np.float32)
    in_maps = []
    for c in range(n_cores):
        in_maps.append(
            {
                "xt": np.ascontiguousarray(xT[:, c * R : (c + 1) * R]).astype(bf),
                "w0t": w0t,
                "w1t": w1t,
                "w2t": w2t,
                "g0": g0,
                "g1": g1,
                "g2": g2,
                "beta2": b2,
            }
        )
    return in_maps, R


def kernel(
    x,
    W0,
    b0,
    gamma0,
    beta0,
    W1,
    b1,
    gamma1,
    beta1,
    W2,
    b2,
    gamma2,
    beta2,
):
    """Full-input entry point: shard across 8 NeuronCores, run, gather.

    b0/b1/b2/beta0/beta1 cancel exactly under training-mode BatchNorm
    (shift invariance), so they are not shipped to the device.
    """
    global LAST_RESULTS
    from concourse.bass_utils import run_bass_kernel_spmd

    x = np.asarray(x, dtype=np.float32)
    B = x.shape[0]
    in_maps, R = prep_inputs(
        x, np.asarray(W0), np.asarray(W1), np.asarray(W2),
        np.asarray(gamma0), np.asarray(gamma1), np.asarray(gamma2),
        np.asarray(beta2),
    )
    nc = _get_program(R, B)
    res = run_bass_kernel_spmd(nc, in_maps, core_ids=list(range(N_CORES)))
    LAST_RESULTS = res
    out = np.empty((B, D_OUT), dtype=np.float32)
    for c in range(N_CORES):
        out[c * R : (c + 1) * R, :] = res.results[c]["out"].T
    return out
